# revision 23
# baseline (speedup 1.0000x reference)
"""DigitCaps (CapsNet dynamic routing) Trainium2 kernel, v2.

Math (per reference):
  u_hat[b,i,o,d] = sum_k W[i,o,d,k] * x[b,i,k]      B=256, IC=1152, K=8, O=10, D=16
  3 routing iters: c = softmax_o(bl); s = sum_i c*u_hat; v = squash(s);
                   bl += sum_d u_hat*v
  out v: [B, 10, 16]

Strategy: pure data-parallel over batch, 8 cores x 32 samples, batch in 4
bgroups of 8. Per bgroup the einsum uses a block-diagonal x operand built
ON-CHIP (DVE mask-multiply from a compact x tile) so HBM traffic stays
small. Partition packing (i16,b8)=128.

Routing:
  it=0: c uniform 0.1 -> s0 = 0.1*sum_i u_hat via 72 PSUM-accum matmuls with
        a constant [128,8] delta(b) lhsT -> ps0 [8,(o,d)], squash directly
        (no diag extraction), vrep via one [8,128] broadcast matmul.
  it>0: softmax on DVE/Scalar; call[p,(g,b',o)] = c * delta(b==b') built in
        ONE broadcast TT (bf16 4x rate); s via 72 matmuls [128,80]x[128,160]
        PSUM-accum; diag(o==o') extract on GpSimd+DVE; squash via
        exp(-0.5*ln(ns+eps) - ln(1+ns)) (keeps Scalar on one act table).
  bl-update: z = u*vrep (TT bf16 4x), one bf16 pair-reduction over d, then
        f32 tensor_reduce -> += bl. f32 accumulation avoids the bf16
        reduce-tree precision loss of v1.
"""

import sys

sys.path.insert(0, "/opt/trn_rl_repo")

import numpy as np
import ml_dtypes

import concourse.bass as bass
import concourse.bacc as bacc_mod
from concourse import mybir
from concourse.tile import TileContext
from concourse.bass_utils import run_bass_kernel_spmd

BF16 = ml_dtypes.bfloat16

# Problem dims (hardcoded per harness contract)
B, IC, KD, OC, OD = 256, 1152, 8, 10, 16
NCORES = 8
BL = B // NCORES          # 32 samples per core
BG = 8                    # bgroup size
NBG = BL // BG            # 4 bgroups
G = IC // 16              # 72 groups of 16 in-caps
ODF = OC * OD             # 160
ITERS = 3
GO = G * OC               # 720 logit columns
GQ = G // 4               # 18 groups per xblk quarter

_BUILT = None


def _consts():
    """Host-side constant tensors shared by all cores."""
    p = np.arange(128)
    c128 = np.arange(128)
    # mask16b [128,128]: delta(p//8 == c//8)  (xblk build)
    mask16b = (p[:, None] // 8 == c128[None, :] // 8).astype(np.float32)
    # mcb8s [128,8]: 0.1*delta(p%8 == c)      (it0 s lhsT)
    mcb8s = 0.1 * (p[:, None] % 8 == np.arange(8)[None, :]).astype(np.float32)
    # mcb [128,80]: delta(p%8 == c//10)       (call build)
    mcb = (p[:, None] % 8 == np.arange(80)[None, :] // 10).astype(np.float32)
    # mcb8T [8,128]: delta(r == c%8)          (it0 vrep lhsT)
    mcb8T = (np.arange(8)[:, None] == c128[None, :] % 8).astype(np.float32)
    # arep [80,128]: delta(r//10 == c%8)      (it>0 vrep lhsT)
    r80 = np.arange(80)
    arep = (r80[:, None] // 10 == c128[None, :] % 8).astype(np.float32)
    # msks_od [80,160] delta(r%10 == c//16)   (vexp build, (o',d) cols)
    c160 = np.arange(160)
    msks_od = (r80[:, None] % 10 == c160[None, :] // 16).astype(np.float32)
    # msks_do [80,160] f32 delta(r%10 == c%10)  (diag extract, (d,o') cols)
    msks_do = (r80[:, None] % 10 == c160[None, :] % 10).astype(np.float32)

    cp128 = np.concatenate([mask16b, mcb8s, mcb], axis=1)      # [128, 216]
    cp80 = np.concatenate([arep, msks_od], axis=1)             # [80, 288]
    return (
        cp128.astype(BF16),
        cp80.astype(BF16),
        msks_do,
        mcb8T.astype(BF16),
    )


def _prep_w(W):
    """wr [128, G*160] bf16: wr[i16*8+k, g*160 + o*16+d] = W[g*16+i16,o,d,k]"""
    wv = W.reshape(G, 16, OC, OD, KD)  # [g, i16, o, d, k]
    wr = wv.transpose(1, 4, 0, 2, 3).reshape(128, G * ODF)
    return np.ascontiguousarray(wr).astype(BF16)


def _prep_core(x_c):
    """xc [128, NBG*576] bf16: xc[(i16,k), bg*576 + g*8 + b] = x[bg*8+b, g*16+i16, k]"""
    xv = x_c.reshape(NBG, BG, G, 16, KD)  # [bg, b, g, i16, k]
    xc = xv.transpose(3, 4, 0, 2, 1).reshape(128, NBG * G * BG)
    return {"xc": np.ascontiguousarray(xc).astype(BF16)}


def make_in_maps(x, W):
    """Per-core DRAM input maps for run_bass_kernel_spmd."""
    cp128, cp80, msksdo, mcb8t = _consts()
    wr = _prep_w(np.asarray(W, np.float32))
    x = np.asarray(x, np.float32)
    in_maps = []
    for c in range(NCORES):
        m = _prep_core(x[c * BL : (c + 1) * BL])
        m.update(wr=wr, cp128=cp128, cp80=cp80, msksdo=msksdo, mcb8t=mcb8t)
        in_maps.append(m)
    return in_maps


def _build():
    global _BUILT
    if _BUILT is not None:
        return _BUILT

    nc = bacc_mod.Bacc()
    dt = mybir.dt
    xc_d = nc.dram_tensor("xc", [128, NBG * G * BG], dt.bfloat16, kind="ExternalInput")
    wr_d = nc.dram_tensor("wr", [128, G * ODF], dt.bfloat16, kind="ExternalInput")
    cp128_d = nc.dram_tensor("cp128", [128, 216], dt.bfloat16, kind="ExternalInput")
    cp80_d = nc.dram_tensor("cp80", [80, 288], dt.bfloat16, kind="ExternalInput")
    msksdo_d = nc.dram_tensor("msksdo", [80, ODF], dt.float32, kind="ExternalInput")
    mcb8t_d = nc.dram_tensor("mcb8t", [8, 128], dt.bfloat16, kind="ExternalInput")
    vout_d = nc.dram_tensor("vout", [BL, OC, OD], dt.float32, kind="ExternalOutput")

    AF = mybir.ActivationFunctionType
    ALU = mybir.AluOpType
    AX = mybir.AxisListType

    with TileContext(nc) as tc:
        with (
            tc.tile_pool(name="consts", bufs=1) as cpool,
            tc.tile_pool(name="wrp", bufs=1) as wpool,
            tc.tile_pool(name="uhp", bufs=1) as uhpool,
            tc.tile_pool(name="blp", bufs=1) as blpool,
            tc.tile_pool(name="xbp", bufs=3) as xpool,
            tc.tile_pool(name="xcp", bufs=1) as xcpool,
            tc.tile_pool(name="callp", bufs=1) as callpool,
            tc.tile_pool(name="zp", bufs=1) as zpool,
            tc.tile_pool(name="smx", bufs=2) as smxpool,
            tc.tile_pool(name="small", bufs=2) as spool,
            tc.tile_pool(name="pe", bufs=2, space="PSUM") as pe_pool,
            tc.tile_pool(name="ps", bufs=2, space="PSUM") as ps_pool,
            tc.tile_pool(name="pv", bufs=1, space="PSUM") as pv_pool,
        ):
            # ---- resident constants (few big DMAs)
            cp128 = cpool.tile([128, 216], dt.bfloat16, tag="cp128")
            nc.sync.dma_start(out=cp128[:], in_=cp128_d[:])
            cp80 = cpool.tile([80, 288], dt.bfloat16, tag="cp80")
            nc.sync.dma_start(out=cp80[:], in_=cp80_d[:])
            msksdo = cpool.tile([80, ODF], dt.float32, tag="msksdo")
            nc.sync.dma_start(out=msksdo[:], in_=msksdo_d[:])
            mcb8t = cpool.tile([8, 128], dt.bfloat16, tag="mcb8t")
            nc.sync.dma_start(out=mcb8t[:], in_=mcb8t_d[:])
            mask16b = cp128[:, 0:128]
            mcb8s = cp128[:, 128:136]
            mcb = cp128[:, 136:216]
            arep = cp80[:, 0:128]
            msks_od = cp80[:, 128:288]

            czero = cpool.tile([128, 1], dt.float32, tag="czero")
            nc.vector.memset(czero[:], 0.0)
            ceps = cpool.tile([80, 1], dt.float32, tag="ceps")
            nc.vector.memset(ceps[:], 1e-8)
            cone = cpool.tile([80, 1], dt.float32, tag="cone")
            nc.vector.memset(cone[:], 1.0)
            mcf = cpool.tile([128, 80], dt.float32, tag="mcf")
            nc.vector.tensor_copy(out=mcf[:], in_=mcb)
            import os as _os0

            _dummy_kb = int(_os0.environ.get("DUMMY_KB", "0"))
            if _dummy_kb:
                dum = cpool.tile(
                    [128, _dummy_kb * 256], dt.float32, tag="dum"
                )
                nc.vector.memset(dum[:, 0:1], 0.0)

            xc_sb = xcpool.tile([128, NBG * G * BG], dt.bfloat16, tag="xc")
            nc.sync.dma_start(out=xc_sb[:], in_=xc_d[:])

            wr_t = []
            for c in range(4):
                wt = wpool.tile([128, 18 * ODF], dt.bfloat16, tag=f"wr{c}")
                nc.sync.dma_start(
                    out=wt[:], in_=wr_d[:, c * 18 * ODF : (c + 1) * 18 * ODF]
                )
                wr_t.append(wt)

            def wr_g(g):
                return wr_t[g // 18][:, (g % 18) * ODF : (g % 18 + 1) * ODF]

            uh_t = [
                uhpool.tile(
                    [128, G * ODF], dt.bfloat16, tag=f"uh{bg}", name=f"uh{bg}"
                )
                for bg in range(NBG)
            ]
            bl_t = [
                blpool.tile(
                    [128, GO], dt.float32, tag=f"bl{bg}", name=f"bl{bg}"
                )
                for bg in range(NBG)
            ]

            # ============ emission helpers ============

            def emit_einsum(bg):
                uh = uh_t[bg]
                for q in range(4):  # quarters of 18 groups
                    xblk = xpool.tile([128, GQ * 128], dt.bfloat16, tag="xblk")
                    # block-diag: xblk[p,(g,i',b)] = xc[p,(g,b)]*mask16b[p,(i',b)]
                    nc.vector.tensor_tensor(
                        out=xblk[:].rearrange(
                            "p (g i b) -> p g i b", i=16, b=BG
                        ),
                        in0=xc_sb[
                            :, bg * 576 + q * GQ * BG : bg * 576 + (q + 1) * GQ * BG
                        ]
                        .rearrange("p (g b) -> p g b", b=BG)
                        .unsqueeze(2)
                        .broadcast_to([128, GQ, 16, BG]),
                        in1=mask16b.rearrange("p (i b) -> p i b", b=BG)
                        .unsqueeze(1)
                        .broadcast_to([128, GQ, 16, BG]),
                        op=ALU.mult,
                    )
                    for gt in range(3):  # 6 g per PSUM tile (2 banks, 3 MM each)
                        pe = pe_pool.tile([128, 1024], dt.float32, tag="pe")
                        for j in range(6):
                            gl = gt * 6 + j          # group within quarter
                            g = q * GQ + gl          # global group
                            off = (j // 3) * 512 + (j % 3) * ODF
                            nc.tensor.matmul(
                                pe[:, off : off + ODF],
                                xblk[:, gl * 128 : (gl + 1) * 128],
                                wr_g(g),
                                start=True,
                                stop=True,
                            )
                        dst = uh[
                            :, (q * 3 + gt) * 6 * ODF : (q * 3 + gt + 1) * 6 * ODF
                        ]
                        src = pe[:].rearrange("p (c f) -> p c f", c=2)[:, :, 0:480]
                        if (q * 3 + gt) < 9:
                            nc.scalar.copy(out=dst, in_=src)
                        else:
                            nc.vector.tensor_copy(out=dst, in_=src)

            def emit_squash(ns, nparts, tag):
                """fac = ns*exp(-0.5*ln(ns+eps)-ln(1+ns)) -> [nparts,w]."""
                w = ns.shape[1]
                L1 = spool.tile([nparts, w], dt.float32, tag=f"L1{tag}")
                nc.scalar.activation(
                    out=L1[:], in_=ns[:], func=AF.Ln, bias=ceps[:nparts]
                )
                L2 = spool.tile([nparts, w], dt.float32, tag=f"L2{tag}")
                nc.scalar.activation(
                    out=L2[:], in_=ns[:], func=AF.Ln, bias=cone[:nparts]
                )
                S = spool.tile([nparts, w], dt.float32, tag=f"S{tag}")
                nc.vector.scalar_tensor_tensor(
                    out=S[:], in0=L1[:], scalar=0.5, in1=L2[:],
                    op0=ALU.mult, op1=ALU.add,
                )
                F = spool.tile([nparts, w], dt.float32, tag=f"F{tag}")
                nc.scalar.activation(
                    out=F[:], in_=S[:], func=AF.Exp, bias=czero[:nparts],
                    scale=-1.0,
                )
                fac = spool.tile([nparts, w], dt.float32, tag=f"fac{tag}")
                nc.gpsimd.tensor_tensor(
                    out=fac[:], in0=ns[:], in1=F[:], op=ALU.mult
                )
                return fac

            it0_state = {}

            def emit_it0_s(bg):
                """it0: s0 = 0.1*sum_i u (72 accum matmuls) + squash -> v0."""
                psf = ps_pool.tile([80, ODF], dt.float32, tag="ps")
                ps0 = psf[0:8, :]
                uh = uh_t[bg]
                for g in range(G):
                    nc.tensor.matmul(
                        ps0,
                        mcb8s,
                        uh[:, g * ODF : (g + 1) * ODF],
                        start=(g == 0),
                        stop=(g == G - 1),
                    )
                s_t0 = spool.tile([8, ODF], dt.float32, tag="s_t0")
                nc.scalar.copy(out=s_t0[:], in_=ps0)
                sq0 = spool.tile([8, ODF], dt.float32, tag="sq0")
                nc.vector.tensor_tensor(
                    out=sq0[:], in0=s_t0[:], in1=s_t0[:], op=ALU.mult
                )
                ns0 = spool.tile([8, OC], dt.float32, tag="ns0")
                nc.vector.tensor_reduce(
                    out=ns0[:],
                    in_=sq0[:].rearrange("p (o d) -> p o d", o=OC),
                    axis=AX.X,
                    op=ALU.add,
                )
                fac0 = emit_squash(ns0, 8, "0")
                v0 = spool.tile([8, ODF], dt.bfloat16, tag="v0")
                nc.vector.tensor_tensor(
                    out=v0[:].rearrange("p (o d) -> p o d", o=OC),
                    in0=s_t0[:].rearrange("p (o d) -> p o d", o=OC),
                    in1=fac0[:].unsqueeze(2).broadcast_to([8, OC, OD]),
                    op=ALU.mult,
                )
                it0_state[bg] = v0

            def emit_vrep_from_v0(bg):
                v0 = it0_state[bg]
                pv = pv_pool.tile([128, ODF], dt.float32, tag="pv")
                nc.tensor.matmul(pv[:], mcb8t[:], v0[:], start=True, stop=True)
                vrep = spool.tile([128, ODF], dt.bfloat16, tag="vrep")
                nc.scalar.copy(out=vrep[:], in_=pv[:])
                return vrep

            def emit_z(bg, vrep, first):
                """bl[bg] (+)= sum_d u*vrep ; chunked in 2 halves of 36 g."""
                uh = uh_t[bg]
                bl = bl_t[bg]
                t1 = None if first else smxpool.tile([128, GO], dt.float32, tag="t1")
                tgt = bl if first else t1
                for h in range(2):
                    z = zpool.tile([128, 36 * ODF], dt.bfloat16, tag="z")
                    nc.vector.tensor_tensor(
                        out=z[:].rearrange("p (g f) -> p g f", f=ODF),
                        in0=uh[:, h * 36 * ODF : (h + 1) * 36 * ODF].rearrange(
                            "p (g f) -> p g f", f=ODF
                        ),
                        in1=vrep[:].unsqueeze(1).broadcast_to([128, 36, ODF]),
                        op=ALU.mult,
                    )
                    z8 = zpool.tile([128, 36 * 80], dt.bfloat16, tag="z8")
                    zv = z[:].rearrange("p (q d) -> p q d", d=OD)
                    nc.vector.tensor_tensor(
                        out=z8[:].rearrange("p (q d) -> p q d", d=8),
                        in0=zv[:, :, 0:8],
                        in1=zv[:, :, 8:16],
                        op=ALU.add,
                    )
                    nc.vector.tensor_reduce(
                        out=tgt[:, h * 360 : (h + 1) * 360],
                        in_=z8[:].rearrange("p (q d) -> p q d", d=8),
                        axis=AX.X,
                        op=ALU.add,
                    )
                if not first:
                    nc.gpsimd.tensor_tensor(
                        out=bl[:], in0=bl[:], in1=t1[:], op=ALU.add
                    )

            call_state = {}

            import os as _os3

            SMX_OPS = int(_os3.environ.get("SMX_OPS", "5"))

            def emit_softmax(bg):
                """c=softmax(bl) over o; build call [128,(g,b',o)] bf16."""
                bl = bl_t[bg]
                ee = smxpool.tile([128, GO], dt.float32, tag="ee")
                if SMX_OPS == 0:
                    nc.scalar.copy(out=ee[:], in_=bl[:])
                else:
                    nc.scalar.activation(
                        out=ee[:], in_=bl[:], func=AF.Exp, bias=czero[:]
                    )
                if SMX_OPS == 0:
                    call = callpool.tile(
                        [128, G * 80], dt.bfloat16, tag="call"
                    )
                    nc.vector.memset(call[:], 0.01)
                    call_state[bg] = call
                    return
                if SMX_OPS < 2:
                    call = callpool.tile(
                        [128, G * 80], dt.bfloat16, tag="call"
                    )
                    nc.vector.memset(call[:], 0.01)
                    call_state[bg] = call
                    return
                zz = spool.tile([128, G], dt.float32, tag="zz")
                nc.vector.tensor_reduce(
                    out=zz[:],
                    in_=ee[:].rearrange("p (g o) -> p g o", o=OC),
                    axis=AX.X,
                    op=ALU.add,
                )
                if SMX_OPS < 3:
                    call = callpool.tile(
                        [128, G * 80], dt.bfloat16, tag="call"
                    )
                    nc.vector.memset(call[:], 0.01)
                    call_state[bg] = call
                    return
                rz = spool.tile([128, G], dt.float32, tag="rz")
                nc.vector.reciprocal(out=rz[:], in_=zz[:])
                if SMX_OPS < 4:
                    call = callpool.tile(
                        [128, G * 80], dt.bfloat16, tag="call"
                    )
                    nc.vector.memset(call[:], 0.01)
                    call_state[bg] = call
                    return
                import os as _os2

                if _os2.environ.get("CC_F32", "0") == "1":
                    cC = spool.tile([128, GO], dt.float32, tag="cC")
                    nc.vector.tensor_tensor(
                        out=cC[:].rearrange("p (g o) -> p g o", o=OC),
                        in0=ee[:].rearrange("p (g o) -> p g o", o=OC),
                        in1=rz[:].unsqueeze(2).broadcast_to([128, G, OC]),
                        op=ALU.mult,
                    )
                    cCb = spool.tile([128, GO], dt.bfloat16, tag="cCb")
                    nc.vector.tensor_copy(out=cCb[:], in_=cC[:])
                else:
                    cCb = spool.tile([128, GO], dt.bfloat16, tag="cCb")
                    nc.vector.tensor_tensor(
                        out=cCb[:].rearrange("p (g o) -> p g o", o=OC),
                        in0=ee[:].rearrange("p (g o) -> p g o", o=OC),
                        in1=rz[:].unsqueeze(2).broadcast_to([128, G, OC]),
                        op=ALU.mult,
                    )
                if SMX_OPS < 5:
                    call = callpool.tile(
                        [128, G * 80], dt.bfloat16, tag="call"
                    )
                    nc.vector.memset(call[:], 0.01)
                    call_state[bg] = call
                    return
                call = callpool.tile([128, G * 80], dt.bfloat16, tag="call")
                import os as _os

                if _os.environ.get("CALL_TS", "0") == "1":
                    callv = call[:].rearrange(
                        "p (g b o) -> p g b o", b=BG, o=OC
                    )
                    for bp in range(BG):
                        nc.vector.tensor_scalar_mul(
                            callv[:, :, bp, :],
                            cCb[:].rearrange("p (g o) -> p g o", o=OC),
                            mcf[:, bp * OC : bp * OC + 1],
                        )
                else:
                    nc.vector.tensor_tensor(
                        out=call[:].rearrange(
                            "p (g b o) -> p g b o", b=BG, o=OC
                        ),
                        in0=cCb[:]
                        .rearrange("p (g o) -> p g o", o=OC)
                        .unsqueeze(2)
                        .broadcast_to([128, G, BG, OC]),
                        in1=mcb.rearrange("p (b o) -> p b o", o=OC)
                        .unsqueeze(1)
                        .broadcast_to([128, G, BG, OC]),
                        op=ALU.mult,
                    )
                call_state[bg] = call

            smm_state = {}

            def emit_smm(bg):
                """s matmuls for it>0: accumulate over 72 groups."""
                call = call_state[bg]
                uh = uh_t[bg]
                ps = ps_pool.tile([80, ODF], dt.float32, tag="ps")
                for g in range(G):
                    nc.tensor.matmul(
                        ps[:],
                        call[:, g * 80 : (g + 1) * 80],
                        uh[:, g * ODF : (g + 1) * ODF],
                        start=(g == 0),
                        stop=(g == G - 1),
                    )
                smm_state[bg] = ps

            sq_state = {}

            def emit_diag_squash(bg, last):
                """diag extract -> s_t [80,16]; squash -> vexp (bf16) or out."""
                ps = smm_state[bg]
                tmp = spool.tile([80, ODF], dt.float32, tag="tmp")
                nc.vector.tensor_tensor(
                    out=tmp[:].rearrange("p (d o) -> p d o", o=OC),
                    in0=ps[:]
                    .rearrange("p (o d) -> p o d", o=OC)
                    .transpose([0, 2, 1]),
                    in1=msksdo[:].rearrange("p (d o) -> p d o", o=OC),
                    op=ALU.mult,
                )
                s_t = spool.tile([80, OD], dt.float32, tag="s_t")
                nc.vector.tensor_reduce(
                    out=s_t[:],
                    in_=tmp[:].rearrange("p (d o) -> p d o", o=OC),
                    axis=AX.X,
                    op=ALU.add,
                )
                sq = spool.tile([80, OD], dt.float32, tag="sq")
                ns = spool.tile([80, 1], dt.float32, tag="ns")
                if _os3.environ.get("USE_TTR", "0") == "1":
                    nc.vector.tensor_tensor_reduce(
                        out=sq[:], in0=s_t[:], in1=s_t[:], scale=1.0,
                        scalar=0.0, op0=ALU.mult, op1=ALU.add,
                        accum_out=ns[:],
                    )
                else:
                    nc.vector.tensor_tensor(
                        out=sq[:], in0=s_t[:], in1=s_t[:], op=ALU.mult
                    )
                    nc.vector.tensor_reduce(
                        out=ns[:], in_=sq[:], axis=AX.X, op=ALU.add
                    )
                fac = emit_squash(ns, 80, "r")
                if last:
                    v_f = spool.tile([80, OD], dt.float32, tag="v_f")
                    nc.vector.tensor_scalar_mul(v_f[:], s_t[:], fac[:])
                    nc.sync.dma_start(
                        out=vout_d[bg * BG : (bg + 1) * BG].rearrange(
                            "b o d -> (b o) d"
                        ),
                        in_=v_f[:],
                    )
                    return
                v_bf = spool.tile([80, OD], dt.bfloat16, tag="v_bf")
                nc.vector.tensor_scalar_mul(v_bf[:], s_t[:], fac[:])
                vexp = spool.tile([80, ODF], dt.bfloat16, tag="vexp")
                nc.gpsimd.tensor_tensor(
                    out=vexp[:].rearrange("p (o d) -> p o d", o=OC),
                    in0=msks_od.rearrange("p (o d) -> p o d", o=OC),
                    in1=v_bf[:].unsqueeze(1).broadcast_to([80, OC, OD]),
                    op=ALU.mult,
                )
                sq_state[bg] = vexp

            def emit_vrep(bg):
                vexp = sq_state[bg]
                pv = pv_pool.tile([128, ODF], dt.float32, tag="pv")
                nc.tensor.matmul(pv[:], arep, vexp[:], start=True, stop=True)
                vrep = spool.tile([128, ODF], dt.bfloat16, tag="vrepr")
                nc.scalar.copy(out=vrep[:], in_=pv[:])
                return vrep

            # ============ orchestration ============
            import os

            LEVEL = int(os.environ.get("BISECT_LEVEL", "4"))

            def emit_dummy_out():
                for bg in range(NBG):
                    v_f = spool.tile([80, OD], dt.float32, tag="v_f")
                    nc.vector.memset(v_f[:], 0.0)
                    nc.sync.dma_start(
                        out=vout_d[bg * BG : (bg + 1) * BG].rearrange(
                            "b o d -> (b o) d"
                        ),
                        in_=v_f[:],
                    )

            # Phase A + it0, software-pipelined
            emit_einsum(0)
            emit_einsum(1)
            if LEVEL >= 2:
                emit_it0_s(0)
            emit_einsum(2)
            if LEVEL >= 2:
                emit_it0_s(1)
                vr0 = emit_vrep_from_v0(0)
            emit_einsum(3)
            if LEVEL < 2:
                emit_dummy_out()
            if LEVEL >= 2:
                emit_it0_s(2)
                vr1 = emit_vrep_from_v0(1)
                emit_z(0, vr0, first=True)
                emit_it0_s(3)
                vr2 = emit_vrep_from_v0(2)
                emit_z(1, vr1, first=True)
                if LEVEL >= 3:
                    emit_softmax(0)
                vr3 = emit_vrep_from_v0(3)
                emit_z(2, vr2, first=True)
                if LEVEL >= 3:
                    emit_softmax(1)
                emit_z(3, vr3, first=True)
                if LEVEL == 2:
                    emit_dummy_out()

            if LEVEL == 31:
                # softmax only
                emit_softmax(2)
                emit_softmax(3)
                emit_dummy_out()

            if LEVEL == 32:
                # + smm + diag extract, raw s_t out (no squash)
                emit_softmax(2)
                emit_softmax(3)
                for bg in range(NBG):
                    emit_smm(bg)
                    ps = smm_state[bg]
                    tmp = spool.tile([80, ODF], dt.float32, tag="tmp")
                    nc.vector.tensor_tensor(
                        out=tmp[:].rearrange("p (d o) -> p d o", o=OC),
                        in0=ps[:]
                        .rearrange("p (o d) -> p o d", o=OC)
                        .transpose([0, 2, 1]),
                        in1=msksdo[:].rearrange("p (d o) -> p d o", o=OC),
                        op=ALU.mult,
                    )
                    s_t = spool.tile([80, OD], dt.float32, tag="s_t")
                    nc.vector.tensor_reduce(
                        out=s_t[:],
                        in_=tmp[:].rearrange("p (d o) -> p d o", o=OC),
                        axis=AX.X,
                        op=ALU.add,
                    )
                    nc.sync.dma_start(
                        out=vout_d[bg * BG : (bg + 1) * BG].rearrange(
                            "b o d -> (b o) d"
                        ),
                        in_=s_t[:],
                    )

            if LEVEL == 3:
                # it1 only, output from its diag path (no vrep/z)
                emit_smm(0)
                emit_softmax(2)
                emit_diag_squash(0, last=True)
                emit_smm(1)
                emit_softmax(3)
                emit_diag_squash(1, last=True)
                emit_smm(2)
                emit_diag_squash(2, last=True)
                emit_smm(3)
                emit_diag_squash(3, last=True)

            if LEVEL == 4:
                # it = 1
                emit_smm(0)
                emit_softmax(2)
                emit_diag_squash(0, last=False)
                emit_smm(1)
                vr0 = emit_vrep(0)
                emit_softmax(3)
                emit_diag_squash(1, last=False)
                emit_smm(2)
                vr1 = emit_vrep(1)
                emit_z(0, vr0, first=False)
                emit_diag_squash(2, last=False)
                emit_smm(3)
                vr2 = emit_vrep(2)
                emit_z(1, vr1, first=False)
                emit_softmax(0)
                emit_diag_squash(3, last=False)
                vr3 = emit_vrep(3)
                emit_z(2, vr2, first=False)
                emit_softmax(1)
                emit_z(3, vr3, first=False)

                # it = 2 (final)
                emit_smm(0)
                emit_softmax(2)
                emit_diag_squash(0, last=True)
                emit_smm(1)
                emit_softmax(3)
                emit_diag_squash(1, last=True)
                emit_smm(2)
                emit_diag_squash(2, last=True)
                emit_smm(3)
                emit_diag_squash(3, last=True)

    nc.finalize()
    _BUILT = nc
    return nc


def kernel(x, W):
    nc = _build()
    in_maps = make_in_maps(x, W)
    res = run_bass_kernel_spmd(nc, in_maps, core_ids=list(range(NCORES)))
    outs = res.results
    v = np.concatenate([np.asarray(o["vout"]) for o in outs], axis=0)
    return v.astype(np.float32)


if __name__ == "__main__":
    rng = np.random.default_rng(0)
    x = rng.standard_normal((B, IC, KD)).astype(np.float32)
    W = rng.standard_normal((IC, OC, OD, KD)).astype(np.float32)
    v = kernel(x, W)
    print("out", v.shape, v.dtype, float(np.abs(v).mean()))
